# revision 7
# baseline (speedup 1.0000x reference)
"""Criss-cross attention (CCNet) kernel for 8 TRN2 NeuronCores.

Data-parallel over batch N=8: one image per core. Per image (512ch, 96x96):
  t/f = 1x1 conv to 64ch; g = 1x1 conv to 512ch
  row/col affinities -> softmax over 191 (96 row + 95 col, col diag excluded)
  weighted row/col aggregation of g -> inc 1x1 conv -> residual add.

All matmuls bf16 operands with f32 PSUM accumulation. Weight transposes and
bf16 casts are done on host (numpy) - they are kernel inputs.
"""

import sys

sys.path.insert(0, "/opt/trn_rl_repo")

from contextlib import ExitStack

import numpy as np
import ml_dtypes

import concourse.bass as bass
import concourse.bacc as bacc
import concourse.tile as tile
from concourse import mybir
from concourse.bass_utils import run_bass_kernel_spmd

BF16 = mybir.dt.bfloat16
F32 = mybir.dt.float32
AF = mybir.ActivationFunctionType

N, C_IN, C_INNER, C_OUT, H, W = 8, 512, 64, 512, 96, 96
HW = H * W  # 9216
KC = C_IN // 128  # 4 contraction chunks

_cache = {}


def build_program():
    nc = bacc.Bacc()

    # ---- DRAM I/O ----
    xbf_d = nc.dram_tensor("x_bf", (128, KC, HW), BF16, kind="ExternalInput")
    xf_d = nc.dram_tensor("x_f32", (KC, 128, HW), F32, kind="ExternalInput")
    tfw_d = nc.dram_tensor("tf_wT", (128, KC, 128), BF16, kind="ExternalInput")
    gw_d = nc.dram_tensor("g_wT", (128, KC, C_OUT), BF16, kind="ExternalInput")
    incw_d = nc.dram_tensor("inc_wT", (128, KC, C_IN), BF16, kind="ExternalInput")
    tfb_d = nc.dram_tensor("tf_b", (128, 1), F32, kind="ExternalInput")
    combb_d = nc.dram_tensor("comb_b", (128, KC), F32, kind="ExternalInput")
    mask_d = nc.dram_tensor("mask", (96, 96), BF16, kind="ExternalInput")
    ones96_d = nc.dram_tensor("ones96", (96, 1), BF16, kind="ExternalInput")
    ones1_d = nc.dram_tensor("ones1", (1, 128), BF16, kind="ExternalInput")
    out_d = nc.dram_tensor("out", (KC, 128, HW), F32, kind="ExternalOutput")

    with ExitStack() as ctx:
        tc = ctx.enter_context(tile.TileContext(nc))
        p0 = ctx.enter_context(tc.tile_pool(name="p0", bufs=1))

        # ---- persistent tiles ----
        Xbf = p0.tile([128, KC, H, W], BF16)  # channel-major image, bf16
        ones96 = p0.tile([96, 1], BF16)
        ones1 = p0.tile([1, 128], BF16)
        mask = p0.tile([96, 96], BF16)
        rbf = p0.tile([1, H, W], BF16)  # 1/denominator, row-major (y, x)

        nc.sync.dma_start(out=Xbf, in_=xbf_d[:].rearrange("p a (h w) -> p a h w", h=H))
        nc.sync.dma_start(out=ones96, in_=ones96_d[:])
        nc.sync.dma_start(out=ones1, in_=ones1_d[:])
        nc.sync.dma_start(out=mask, in_=mask_d[:])

        # T/F (phase 1) and U (phases 3-4) share one big slot: disjoint lifetimes
        TFb = p0.tile([64, 2, H, W], BF16, tag="big", name="TFb")
        T = TFb[:, 0]
        F = TFb[:, 1]

        with tc.tile_pool(name="pw", bufs=1) as pw:
            # exp(affinity) buffers: Wr[i, y, x] (row), Wc[j, x, y] (col)
            Wr = pw.tile([96, H, W], BF16)
            Wc = pw.tile([96, W, H], BF16)

            # ---- phase 1: t/f conv + affinities ----
            with tc.tile_pool(name="pe", bufs=1) as pe, \
                 tc.tile_pool(name="pe_ps", bufs=2, space="PSUM") as pe_ps, \
                 tc.tile_pool(name="ptf_ps", bufs=2, space="PSUM") as ptf_ps:
                tfw = pe.tile([128, KC, 128], BF16)
                tb = pe.tile([64, 1], F32)
                fb = pe.tile([64, 1], F32)
                nc.sync.dma_start(out=tfw, in_=tfw_d[:])
                nc.sync.dma_start(out=tb, in_=tfb_d[0:64])
                nc.sync.dma_start(out=fb, in_=tfb_d[64:128])

                Xflat = Xbf.rearrange("p a h w -> p a (h w)")
                Tflat = TFb.rearrange("p c h w -> p c (h w)")
                for b in range(HW // 512):
                    sl = slice(b * 512, (b + 1) * 512)
                    pst = ptf_ps.tile([64, 512], F32, tag="pt")
                    psf = ptf_ps.tile([64, 512], F32, tag="pf")
                    for k in range(KC):
                        nc.tensor.matmul(
                            pst, tfw[:, k, 0:64], Xflat[:, k, sl],
                            start=(k == 0), stop=(k == KC - 1))
                    for k in range(KC):
                        nc.tensor.matmul(
                            psf, tfw[:, k, 64:128], Xflat[:, k, sl],
                            start=(k == 0), stop=(k == KC - 1))
                    nc.scalar.activation(Tflat[:, 0, sl], pst, AF.Identity, bias=tb)
                    nc.scalar.activation(Tflat[:, 1, sl], psf, AF.Identity, bias=fb)

                # row affinities: E[i, x] = sum_c f[c,y,i] t[c,y,x]
                for y in range(H):
                    ps = pe_ps.tile([96, 96], F32, tag="pe")
                    nc.tensor.matmul(ps, F[:, y, :], T[:, y, :], start=True, stop=True)
                    nc.scalar.activation(Wr[:, y, :], ps, AF.Exp)
                # col affinities: E[j, y] = sum_c f[c,j,x] t[c,y,x]; kill j==y
                for x in range(W):
                    ps = pe_ps.tile([96, 96], F32, tag="pe")
                    nc.tensor.matmul(ps, F[:, :, x], T[:, :, x], start=True, stop=True)
                    nc.scalar.activation(Wc[:, x, :], ps, AF.Exp)
                    nc.vector.tensor_mul(Wc[:, x, :], Wc[:, x, :], mask)

            # ---- phase 2: denominators + normalize weights ----
            WcT = Wc.rearrange("j x y -> j y x")
            with tc.tile_pool(name="pd_ps", bufs=3, space="PSUM") as pd_ps:
                for b in range(24):  # blocks of 4 rows
                    ys = slice(b * 4, (b + 1) * 4)
                    ps = pd_ps.tile([1, 4 * 96], F32, tag="pd")
                    nc.tensor.matmul(ps, ones96, Wr[:, ys, :], start=True, stop=False)
                    nc.tensor.matmul(ps, ones96, WcT[:, ys, :], start=False, stop=True)
                    with nc.allow_low_precision(reason="1/denom in bf16 is fine"):
                        nc.vector.reciprocal(rbf[:, ys, :], ps)

            rflat = rbf.rearrange("p h w -> p (h w)")
            rcm = rbf.rearrange("p h w -> p w h")
            Wrflat = Wr.rearrange("p h w -> p (h w)")
            Wcflat = Wc.rearrange("p w h -> p (w h)")
            with tc.tile_pool(name="pr_ps", bufs=3, space="PSUM") as pr_ps:
                for b in range(HW // 512):  # scale row weights
                    sl = slice(b * 512, (b + 1) * 512)
                    ps = pr_ps.tile([128, 512], F32, tag="pr")
                    nc.tensor.matmul(ps, ones1, rflat[:, sl], start=True, stop=True)
                    nc.vector.tensor_mul(Wrflat[:, sl], Wrflat[:, sl], ps[0:96, :])
                for b in range(24):  # scale col weights (col-major blocks)
                    sl = slice(b * 384, (b + 1) * 384)
                    ps = pr_ps.tile([128, 384], F32, tag="pr2")
                    nc.tensor.matmul(ps, ones1, rcm[:, b * 4:(b + 1) * 4, :],
                                     start=True, stop=True)
                    nc.vector.tensor_mul(Wcflat[:, sl], Wcflat[:, sl], ps[0:96, :])

            # ---- phase 3: g generation + weighted aggregation ----
            U = p0.tile([128, KC, H, W], BF16, tag="big", name="U")
            with tc.tile_pool(name="pu", bufs=4) as pu, \
                 tc.tile_pool(name="pg_ps", bufs=3, space="PSUM") as pg_ps, \
                 tc.tile_pool(name="pu_ps", bufs=4, space="PSUM") as pu_ps:
                gw = pu.tile([128, KC, C_OUT], BF16, tag="gw", bufs=1)
                nc.sync.dma_start(out=gw, in_=gw_d[:])
                # row pass: 4 rows per group
                for y0 in range(0, H, 4):
                    gts = []
                    for r in range(4):
                        psg = pg_ps.tile([96, C_OUT], F32, tag="pg")
                        for k in range(KC):
                            nc.tensor.matmul(psg, Xbf[:, k, y0 + r, :], gw[:, k, :],
                                             start=(k == 0), stop=(k == KC - 1))
                        gt = pu.tile([96, C_OUT], BF16, tag="gt")
                        nc.scalar.activation(gt, psg, AF.Copy)
                        gts.append(gt)
                    for cc in range(4):
                        psu = pu_ps.tile([128, 4 * 96], F32, tag="pu")
                        for r in range(4):
                            nc.tensor.matmul(
                                psu[:, r * 96:(r + 1) * 96],
                                gts[r][:, cc * 128:(cc + 1) * 128],
                                Wr[:, y0 + r, :], start=True, stop=True)
                        nc.vector.tensor_copy(U[:, cc, y0:y0 + 4, :], psu)
                # col pass: 4 cols per group
                for x0 in range(0, W, 4):
                    gts = []
                    for r in range(4):
                        psg = pg_ps.tile([96, C_OUT], F32, tag="pg")
                        for k in range(KC):
                            nc.tensor.matmul(psg, Xbf[:, k, :, x0 + r], gw[:, k, :],
                                             start=(k == 0), stop=(k == KC - 1))
                        gt = pu.tile([96, C_OUT], BF16, tag="gt")
                        nc.scalar.activation(gt, psg, AF.Copy)
                        gts.append(gt)
                    for cc in range(4):
                        psu = pu_ps.tile([128, 4, 96], F32, tag="pu")
                        for r in range(4):
                            nc.tensor.matmul(
                                psu[:, r, :],
                                gts[r][:, cc * 128:(cc + 1) * 128],
                                Wc[:, x0 + r, :], start=True, stop=True)
                        uv = U[:, cc, :, x0:x0 + 4]
                        nc.vector.tensor_add(uv, uv, psu.rearrange("p x y -> p y x"))

        # ---- phase 4: inc conv + bias + residual ----
        Uflat = U.rearrange("p a h w -> p a (h w)")
        with tc.tile_pool(name="pi", bufs=1) as pi, \
             tc.tile_pool(name="pix", bufs=3) as pix, \
             tc.tile_pool(name="po_ps", bufs=2, space="PSUM") as po_ps:
            incw = pi.tile([128, KC, C_IN], BF16)
            combb = pi.tile([128, KC], F32)
            nc.sync.dma_start(out=incw, in_=incw_d[:])
            nc.sync.dma_start(out=combb, in_=combb_d[:])
            for c2 in range(KC):
                for b in range(HW // 512):
                    sl = slice(b * 512, (b + 1) * 512)
                    ps = po_ps.tile([128, 512], F32, tag="po")
                    for k in range(KC):
                        nc.tensor.matmul(ps, incw[:, k, c2 * 128:(c2 + 1) * 128],
                                         Uflat[:, k, sl],
                                         start=(k == 0), stop=(k == KC - 1))
                    xr = pix.tile([128, 512], F32, tag="xr")
                    nc.sync.dma_start(out=xr, in_=xf_d[c2][:, sl])
                    ot = pix.tile([128, 512], F32, tag="ot")
                    nc.scalar.activation(ot, ps, AF.Identity, bias=combb[:, c2:c2 + 1])
                    nc.vector.tensor_add(ot, ot, xr)
                    nc.sync.dma_start(out=out_d[c2][:, sl], in_=ot)

    nc.finalize()
    return nc


def _prep_shared(t_w, t_b, f_w, f_b, g_w, g_b, inc_w, inc_b):
    bf = ml_dtypes.bfloat16
    tf_wT = np.concatenate([t_w.T, f_w.T], axis=1)  # (512, 128)
    d = {
        "tf_wT": np.ascontiguousarray(
            tf_wT.reshape(KC, 128, 128).transpose(1, 0, 2)).astype(bf),
        "g_wT": np.ascontiguousarray(
            g_w.T.reshape(KC, 128, C_OUT).transpose(1, 0, 2)).astype(bf),
        "inc_wT": np.ascontiguousarray(
            inc_w.T.reshape(KC, 128, C_IN).transpose(1, 0, 2)).astype(bf),
        "tf_b": np.concatenate([t_b, f_b]).reshape(128, 1).astype(np.float32),
        "comb_b": np.ascontiguousarray(
            (inc_b + inc_w @ g_b).reshape(KC, 128).T).astype(np.float32),
        "mask": (1.0 - np.eye(96)).astype(bf),
        "ones96": np.ones((96, 1), bf),
        "ones1": np.ones((1, 128), bf),
    }
    return d


def kernel(x, t_w, t_b, f_w, f_b, g_w, g_b, inc_w, inc_b):
    x = np.asarray(x, dtype=np.float32)
    shared = _prep_shared(
        np.asarray(t_w, np.float32), np.asarray(t_b, np.float32),
        np.asarray(f_w, np.float32), np.asarray(f_b, np.float32),
        np.asarray(g_w, np.float32), np.asarray(g_b, np.float32),
        np.asarray(inc_w, np.float32), np.asarray(inc_b, np.float32))

    bf = ml_dtypes.bfloat16
    in_maps = []
    for n in range(N):
        xi = x[n].reshape(KC, 128, HW)  # (4, 128, 9216)
        m = dict(shared)
        m["x_f32"] = np.ascontiguousarray(xi)
        m["x_bf"] = np.ascontiguousarray(xi.transpose(1, 0, 2)).astype(bf)
        in_maps.append(m)

    if "nc" not in _cache:
        _cache["nc"] = build_program()
    res = run_bass_kernel_spmd(_cache["nc"], in_maps, core_ids=list(range(N)))
    out = np.stack([r["out"].reshape(C_IN, H, W) for r in res.results])
    return out.astype(np.float32)


if __name__ == "__main__":
    rng = np.random.default_rng(0)
    ins = {
        "x": rng.standard_normal((N, C_IN, H, W), dtype=np.float32),
        "t_w": rng.standard_normal((C_INNER, C_IN), dtype=np.float32) * 0.02,
        "t_b": np.zeros(C_INNER, np.float32),
        "f_w": rng.standard_normal((C_INNER, C_IN), dtype=np.float32) * 0.02,
        "f_b": np.zeros(C_INNER, np.float32),
        "g_w": rng.standard_normal((C_OUT, C_IN), dtype=np.float32) * 0.02,
        "g_b": np.zeros(C_OUT, np.float32),
        "inc_w": rng.standard_normal((C_IN, C_OUT), dtype=np.float32) * 0.02,
        "inc_b": np.zeros(C_IN, np.float32),
    }
    y = kernel(**ins)
    print(y.shape, y.dtype)
